# revision 1
# baseline (speedup 1.0000x reference)
"""Multi-head attention (B=2, S=2048, D=1024, H=16) on 8 Trainium2 cores.

Sharding: core = 4*b + g  (b = batch 0..1, g = head-group 0..3, 4 heads each).
Each core computes, for its batch b and head-group g (256 of the 1024 dims):
  QT/KT = (x @ W^T)^T  in [d, s] layout   (d on partitions)
  V     = x @ W^T      in [s, d] layout   (s on partitions)
  ST    = scores^T     in [k, q] layout   (k on partitions)  -> exp on ACT
  U     = V^T @ P^T    in [d, q] layout + per-head denominators Z via ones-matmul
  UN    = U / Z        (PE-broadcast reciprocal, DVE multiply)
  Ypart = UN^T @ WoT   in [q, e] layout   (partial over this group's 256 dims)
Host sums the 4 per-group partials per batch and adds b_o.

All matmuls run in bfloat16 (1 cycle/row, FWL weight loads).
"""

import os
from contextlib import ExitStack

import ml_dtypes
import numpy as np

import concourse.bass as bass
import concourse.tile as tile
from concourse import bacc, mybir
from concourse.tile import add_dep_helper

B, S, D = 2, 2048, 1024
H, DH = 16, 64
NCORES = 8
NG = 4                  # head-group shards
DG = D // NG            # 256 dims per head-group (4 heads)
P = 128
QC = 512                # q-chunk width
NQC = S // QC           # 4
NKT = S // P            # 16 k-tiles of 128
CD = D // P             # 8 contraction tiles for the projections
F32 = mybir.dt.float32
BF16 = mybir.dt.bfloat16
AF = mybir.ActivationFunctionType
SCALE = 1.0 / float(np.sqrt(D))





def _body(ctx: ExitStack, tc: "tile.TileContext", io: dict):
    nc = tc.nc
    # bf16 operands feed the PE at full rate (1 cycle/row + fast weight load);
    # accumulation stays fp32 in PSUM.
    ctx.enter_context(nc.allow_low_precision(reason="bf16 matmul pipeline"))
    sb = ctx.enter_context(tc.tile_pool(name="sb", bufs=1))

    # --- constants (memset can't target f32r; DMA from a ones input) -------
    ones_col = sb.tile([1, P], BF16, tag="ones_col", bufs=1, name="ones_col")
    nc.sync.dma_start(ones_col[:], io["ones"][None, :])

    # --- biases ------------------------------------------------------------
    bq = sb.tile([P, 2], F32, tag="bq", bufs=1, name="bq")
    nc.sync.dma_start(bq[:], io["bq"].rearrange("(t p) -> p t", p=P))
    bk = sb.tile([P, 2], F32, tag="bk", bufs=1, name="bk")
    nc.sync.dma_start(bk[:], io["bk"].rearrange("(t p) -> p t", p=P))
    bv_row = sb.tile([1, DG], BF16, tag="bv", bufs=1, name="bv_row")
    nc.sync.dma_start(bv_row[:], io["bv"][None, :])

    # --- output projection weights -----------------------------------------
    woT = []
    for pr in range(2):
        t = sb.tile([P, D], BF16, tag="wo", bufs=2, name=f"woT{pr}")
        nc.sync.dma_start(t[:], io["wo"][pr * P : (pr + 1) * P, :])
        woT.append(t)

    # --- phase 1: projections (own PSUM pool, 8 banks) ---------------------
    # Order Q -> V -> K: attention's PSUM pool allocation waits for this
    # pool's release, so the last projection should be the one attention
    # needs first (K chunk 0 for the first scores).
    QT, KT = {}, {}
    V = {}

    def qk_proj(ps1, nm, xkey, wkey, bias, outmap):
        w = sb.tile([P, CD, DG], BF16, tag="w", bufs=2, name=f"w{nm}")
        nc.sync.dma_start(w[:], io[wkey].rearrange("(c p) d -> p c d", p=P))
        psg = {}
        for d in range(2):
            for sc in range(NQC):
                psg[d, sc] = ps1.tile(
                    [P, QC], F32, tag="proj", bufs=8, name=f"ps_{nm}{d}{sc}"
                )
        for c in range(CD):
            xt = sb.tile([P, S], BF16, tag="x", bufs=8, name=f"x{nm}{c}")
            nc.sync.dma_start(xt[:], io[xkey][c * P : (c + 1) * P, :])
            for d in range(2):
                for sc in range(NQC):
                    nc.tensor.matmul(
                        psg[d, sc][:],
                        (w[:, c, d * P : (d + 1) * P]),
                        (xt[:, sc * QC : (sc + 1) * QC]),
                        start=(c == 0),
                        stop=(c == CD - 1),
                    )
        for d in range(2):
            for sc in range(NQC):
                t = sb.tile([P, QC], BF16, tag=f"{nm}t", bufs=8, name=f"{nm}T{d}{sc}")
                nc.vector.tensor_scalar_add(t[:], psg[d, sc][:], bias[:, d : d + 1])
                outmap[d, sc] = t

    with tc.tile_pool(name="ps_proj", bufs=1, space="PSUM") as ps1:
        qk_proj(ps1, "q", "xq", "wq", bq, QT)

        # V projection: V [2048, 256] as 16 tiles of [128, 256]; bias b_v is
        # folded in by seeding each PSUM accumulation with ones_col^T @ bv_row.
        wv = sb.tile([P, CD, DG], BF16, tag="w", bufs=2, name="wv")
        nc.sync.dma_start(wv[:], io["wv"].rearrange("(c p) d -> p c d", p=P))
        psv = {
            sp: ps1.tile([P, 2, DG], F32, tag="proj", bufs=8, name=f"psv{sp}")
            for sp in range(8)
        }
        for sp in range(8):
            seed = None
            for j in range(2):
                mm = nc.tensor.matmul(
                    psv[sp][:, j, :],
                    (ones_col[:, 0:P]),
                    (bv_row[:]),
                    start=(j == 0),
                    stop=False,
                )
                # start=True must execute before any other matmul in the bank;
                # disjoint-slice writes carry no natural dep, so add one.
                if j == 0:
                    seed = mm
                else:
                    add_dep_helper(mm.ins, seed.ins, reason="psum group order")
        last_j0 = {}
        for c in range(CD):
            xt = sb.tile([P, S], BF16, tag="x", bufs=8, name=f"xv{c}")
            nc.sync.dma_start(xt[:], io["xv"][c * P : (c + 1) * P, :])
            for sp in range(8):
                for j in range(2):
                    st_i = sp * 2 + j
                    mm = nc.tensor.matmul(
                        psv[sp][:, j, :],
                        (xt[:, st_i * P : (st_i + 1) * P]),
                        (wv[:, c, :]),
                        start=False,
                        stop=(c == CD - 1 and j == 1),
                    )
                    if j == 0:
                        last_j0[sp] = mm
                    elif c == CD - 1:
                        # stop=True closes the whole bank's group; it must run
                        # after the other slice's last matmul.
                        add_dep_helper(mm.ins, last_j0[sp].ins, reason="psv stop order")
        # V_aug tiles [128, 4, 65]: per head 64 V columns + a ones column that
        # accumulates the softmax denominator into row 64 of U_h.
        ones4 = sb.tile([P, 4], BF16, tag="ones4", bufs=1, name="ones4")
        nc.sync.dma_start(ones4[:], io["ones4"][:])
        for sp in range(8):
            for j in range(2):
                vt = sb.tile([P, 4, DH + 1], BF16, tag="v", bufs=16, name=f"V{sp}_{j}")
                nc.vector.tensor_copy(
                    vt[:, :, 0:DH],
                    psv[sp][:, j, :].rearrange("p (g d) -> p g d", g=4),
                )
                nc.vector.tensor_copy(vt[:, :, DH : DH + 1], ones4[:, :, None])
                V[sp * 2 + j] = vt

        qk_proj(ps1, "k", "xk", "wk", bk, KT)

    # --- attention, per q-chunk --------------------------------------------
    # Per-head PSUM accumulators U_h [65, 512]: rows 0..63 are sum_k P*V, row
    # 64 is the softmax denominator (from V_aug's ones column). All matmul
    # outputs start at partition 0 (col-offset tile_position fails walrus
    # codegen in this toolchain).
    ps2 = ctx.enter_context(tc.tile_pool(name="ps_attn", bufs=1, space="PSUM"))
    UN = {}
    YSB = {}
    pending = []

    def emit_outproj_unit():
        if not pending:
            return
        qcp, qi, ec = pending.pop(0)
        qt = qcp * 4 + qi
        if ec == 0:
            YSB[qt] = sb.tile([P, D], F32, tag="y", bufs=6, name=f"Y{qt}")
        ysb = YSB[qt]
        yps = ps2.tile([P, QC], F32, tag="st", bufs=3, name=f"yp{qt}_{ec}")
        for pr in range(2):
            nc.tensor.matmul(
                yps[:],
                (UN[qcp, pr][:, qi * P : (qi + 1) * P]),
                (woT[pr][:, ec * QC : (ec + 1) * QC]),
                start=(pr == 0),
                stop=(pr == 1),
            )
        nc.vector.tensor_copy(ysb[:, ec * QC : (ec + 1) * QC], yps[:])
        if ec == 1:
            nc.sync.dma_start(io["y"][qt * P : (qt + 1) * P, :], ysb[:])

    for qc in range(NQC):
        # Head-pairs are processed in serial k-sweeps: only 2 U accumulator
        # banks live at a time, which frees PSUM for 3 ST slots (6 banks) so
        # the PE can run further ahead of the exp pipeline and stay warm.
        # Pair-0 normalization overlaps pair-1's k-sweep.
        for pair in range(2):
            heads = (2 * pair, 2 * pair + 1)
            U = {
                h: ps2.tile([P, QC], F32, tag="u", bufs=2, name=f"U{qc}_{h}")
                for h in heads
            }
            for kg in range(NKT // 2):
                for h in heads:
                    pr, lo = h // 2, (h % 2) * 64
                    st2 = ps2.tile(
                        [P, 2, QC], F32, tag="st", bufs=3, name=f"st{qc}_{kg}_{h}"
                    )
                    for kk in range(2):
                        k_tile = kg * 2 + kk
                        sc, off = divmod(k_tile, 4)
                        nc.tensor.matmul(
                            st2[:, kk, :],
                            (KT[pr, sc][lo : lo + 64, off * P : (off + 1) * P]),
                            (QT[pr, qc][lo : lo + 64, :]),
                            start=True,
                            stop=True,
                            tile_position=(lo, 0),
                        )
                    pt2 = sb.tile(
                        [P, 2, QC], BF16, tag="pt", bufs=8, name=f"pt{qc}_{kg}_{h}"
                    )
                    nc.scalar.activation(pt2[:], st2[:], AF.Exp, scale=SCALE)
                    for kk in range(2):
                        k_tile = kg * 2 + kk
                        nc.tensor.matmul(
                            U[h][0:65, :],
                            (V[k_tile][:, h, :]),
                            (pt2[:, kk, :]),
                            start=(kg == 0 and kk == 0),
                            stop=(kg == NKT // 2 - 1 and kk == 1),
                        )
                # one out-projection unit of a previous q-chunk every other
                # k-group: independent PE filler while ACT runs exp.
                if kg % 2 == pair:
                    emit_outproj_unit()

            # normalize this pair: UN rows = U_h[0:64] * (1/Z_h); the odd
            # head's rows are DMA-shifted into partitions 64..127.
            UN[qc, pair] = sb.tile(
                [P, QC], BF16, tag="un", bufs=8, name=f"UN{qc}_{pair}"
            )
            z2 = sb.tile([2, QC], F32, tag="z4", bufs=3, name=f"z2_{qc}_{pair}")
            for i, h in enumerate(heads):
                zs = sb.tile([65, QC], F32, tag="zs", bufs=3, name=f"zs{qc}_{h}")
                nc.vector.tensor_copy(zs[64:65, :], U[h][64:65, :])
                nc.sync.dma_start(z2[i : i + 1, :], zs[64:65, :])
            rz2 = sb.tile([2, QC], F32, tag="rz4", bufs=3, name=f"rz2_{qc}_{pair}")
            nc.vector.reciprocal(rz2[:], z2[:])
            for i, h in enumerate(heads):
                off = (h % 2) * 64
                if i == 0:
                    r0 = rz2[0:1, :]
                else:
                    r0t = sb.tile([1, QC], F32, tag="r0", bufs=3, name=f"r0_{qc}_{h}")
                    nc.sync.dma_start(r0t[:], rz2[1:2, :])
                    r0 = r0t[:]
                rb = sb.tile([64, QC], F32, tag="rb", bufs=4, name=f"rb{qc}_{h}")
                nc.gpsimd.partition_broadcast(rb[:], r0, channels=64)
                if off == 0:
                    nc.vector.tensor_mul(UN[qc, pair][0:64, :], U[h][0:64, :], rb[:])
                else:
                    tmp = sb.tile(
                        [64, QC], BF16, tag="untmp", bufs=3, name=f"untmp{qc}_{h}"
                    )
                    nc.vector.tensor_mul(tmp[:], U[h][0:64, :], rb[:])
                    nc.sync.dma_start(UN[qc, pair][64:128, :], tmp[:])

        pending.extend((qc, qi, ec) for qi in range(4) for ec in range(2))

    while pending:
        emit_outproj_unit()


def build_program():
    nc = bacc.Bacc(
        "TRN2", target_bir_lowering=False, debug=False, num_devices=NCORES
    )
    io = {
        "xq": nc.dram_tensor("xq", [D, S], BF16, kind="ExternalInput").ap(),
        "xk": nc.dram_tensor("xk", [D, S], BF16, kind="ExternalInput").ap(),
        "xv": nc.dram_tensor("xv", [D, S], BF16, kind="ExternalInput").ap(),
        "wq": nc.dram_tensor("wq", [D, DG], BF16, kind="ExternalInput").ap(),
        "wk": nc.dram_tensor("wk", [D, DG], BF16, kind="ExternalInput").ap(),
        "wv": nc.dram_tensor("wv", [D, DG], BF16, kind="ExternalInput").ap(),
        "wo": nc.dram_tensor("wo", [DG, D], BF16, kind="ExternalInput").ap(),
        "bq": nc.dram_tensor("bq", [DG], F32, kind="ExternalInput").ap(),
        "bk": nc.dram_tensor("bk", [DG], F32, kind="ExternalInput").ap(),
        "bv": nc.dram_tensor("bv", [DG], BF16, kind="ExternalInput").ap(),
        "ones": nc.dram_tensor("ones", [P], BF16, kind="ExternalInput").ap(),
        "ones4": nc.dram_tensor("ones4", [P, 4], BF16, kind="ExternalInput").ap(),
        "y": nc.dram_tensor("y", [S, D], F32, kind="ExternalOutput").ap(),
    }
    with tile.TileContext(nc) as tc:
        with ExitStack() as ctx:
            _body(ctx, tc, io)
    nc.compile()
    return nc


_CACHE = {}


def _get_program():
    if "nc" not in _CACHE:
        _CACHE["nc"] = build_program()
    return _CACHE["nc"]


def make_in_maps(inputs):
    q = np.asarray(inputs["query"], np.float32)
    k = np.asarray(inputs["key"], np.float32)
    v = np.asarray(inputs["value"], np.float32)
    W_q = np.asarray(inputs["W_q"], np.float32)
    W_k = np.asarray(inputs["W_k"], np.float32)
    W_v = np.asarray(inputs["W_v"], np.float32)
    W_o = np.asarray(inputs["W_o"], np.float32)
    b_q = np.asarray(inputs["b_q"], np.float32)
    b_k = np.asarray(inputs["b_k"], np.float32)
    b_v = np.asarray(inputs["b_v"], np.float32)

    bf = ml_dtypes.bfloat16
    xT = [
        [np.ascontiguousarray(x[b].T).astype(bf) for b in range(B)]
        for x in (q, k, v)
    ]
    in_maps = []
    for core in range(NCORES):
        b, g = divmod(core, NG)
        sl = slice(g * DG, (g + 1) * DG)
        in_maps.append(
            {
                "xq": xT[0][b],
                "xk": xT[1][b],
                "xv": xT[2][b],
                "wq": np.ascontiguousarray(W_q[sl, :].T).astype(bf),
                "wk": np.ascontiguousarray(W_k[sl, :].T).astype(bf),
                "wv": np.ascontiguousarray(W_v[sl, :].T).astype(bf),
                "wo": np.ascontiguousarray(W_o[:, sl].T).astype(bf),
                "bq": np.ascontiguousarray(b_q[sl]),
                "bk": np.ascontiguousarray(b_k[sl]),
                "bv": np.ascontiguousarray(b_v[sl]).astype(bf),
                "ones": np.ones(P, bf),
                "ones4": np.ones((P, 4), bf),
            }
        )
    return in_maps


def kernel(**inputs):
    from concourse.bass_utils import run_bass_kernel_spmd

    nc = _get_program()
    in_maps = make_in_maps(inputs)
    trace = bool(int(os.environ.get("MHA_TRACE", "0")))
    res = run_bass_kernel_spmd(nc, in_maps, list(range(NCORES)), trace=trace)
    _CACHE["last_results"] = res

    b_o = np.asarray(inputs["b_o"], np.float32)
    out = np.zeros((B, S, D), np.float32)
    for core in range(NCORES):
        b = core // NG
        out[b] += res.results[core]["y"]
    out += b_o[None, None, :]
    return out



# revision 6
# speedup vs baseline: 1.1743x; 1.1743x over previous
"""Multi-head attention (B=2, S=2048, D=1024, H=16) on 8 Trainium2 cores.

Sharding: core = 4*b + g  (b = batch 0..1, g = head-group 0..3, 4 heads each).

Single fused pipeline per core (no serial phases):
  - inputs x are host-relaid to s-major 1MB blocks [sb, p, c, s] so each
    projection chunk / V k-tile is computable as soon as its block lands;
  - attention runs per (q-chunk, head-pair) k-sweeps:
      ST  = scores^T [k, q] via 64-contraction matmuls, the two heads of a
            pair issued adjacently on PE row-groups (0,0)/(64,0) so they
            execute concurrently (~2x);
      exp on ACT (the steady-state bottleneck: 128 instrs x ~1.07us);
      PV  accumulates U_h [65, q] in PSUM (row 64 = softmax denominator
            via a ones column in V_aug);
  - projection / out-projection units are interleaved as PE filler so the
    tensor engine never idles (keeps the HAM clock-gate at 8/8);
  - U is evacuated to SBUF right after each sweep so the 2 U PSUM banks
    recycle; reciprocals batched per q-chunk ([4, 512] in one DVE instr);
  - V bias is folded out algebraically (P@(V + 1 b_v^T)/Z = P@V/Z + 1 b_v^T),
    host adds (W_o @ b_v + b_o) once; y partials stored bf16.

All matmuls in bfloat16 (1 cycle/row, FWL weight loads); fp32 PSUM accum.
"""

import os
from collections import deque
from contextlib import ExitStack

import ml_dtypes
import numpy as np

import concourse.bass as bass
import concourse.tile as tile
from concourse import bacc, mybir

B, S, D = 2, 2048, 1024
H, DH = 16, 64
NCORES = 8
NG = 4                  # head-group shards
DG = D // NG            # 256 dims per head-group (4 heads)
P = 128
QC = 512                # q-chunk width
NQC = S // QC           # 4
NKT = S // P            # 16 k-tiles of 128
NSB = S // QC           # 4 s-blocks per input
CD = D // P             # 8 contraction chunks
F32 = mybir.dt.float32
BF16 = mybir.dt.bfloat16
AF = mybir.ActivationFunctionType
SCALE = 1.0 / float(np.sqrt(D))


def _body(ctx: ExitStack, tc: "tile.TileContext", io: dict):
    nc = tc.nc
    ctx.enter_context(nc.allow_low_precision(reason="bf16 matmul pipeline"))
    sb = ctx.enter_context(tc.tile_pool(name="sb", bufs=1))
    ps = ctx.enter_context(tc.tile_pool(name="ps", bufs=1, space="PSUM"))

    # ---- input DMAs, in bandwidth-priority order -------------------------
    def ldma(nm, shape, dt, src):
        t = sb.tile(shape, dt, tag=nm, bufs=1, name=nm)
        nc.sync.dma_start(t[:], src)
        return t

    wk = ldma("wk", [P, CD, DG], BF16, io["wk"])
    bk = ldma("bk", [P, 2], F32, io["bk"].rearrange("(t p) -> p t", p=P))
    bq = ldma("bq", [P, 2], F32, io["bq"].rearrange("(t p) -> p t", p=P))
    ones4 = ldma("ones4", [P, 4], BF16, io["ones4"])
    xk_t = [ldma(f"xk{b_}", [P, CD, QC], BF16, io["xk"][b_]) for b_ in range(1)]
    wq = ldma("wq", [P, CD, DG], BF16, io["wq"])
    xq_t = [ldma(f"xq{b_}", [P, CD, QC], BF16, io["xq"][b_]) for b_ in range(1)]
    for b_ in range(1, NSB):
        xk_t.append(ldma(f"xk{b_}", [P, CD, QC], BF16, io["xk"][b_]))
    wv = ldma("wv", [P, CD, DG], BF16, io["wv"])
    xv_t = [ldma(f"xv{b_}", [P, CD, QC], BF16, io["xv"][b_]) for b_ in range(NSB)]
    for b_ in range(1, NSB):
        xq_t.append(ldma(f"xq{b_}", [P, CD, QC], BF16, io["xq"][b_]))
    woT = [
        ldma(f"wo{pr}", [P, D], BF16, io["wo"][pr * P : (pr + 1) * P, :])
        for pr in range(2)
    ]

    # ---- ACT table preload + PE HAM warmup -------------------------------
    # Tiny exp on the first-arriving tile triggers the one-time ~2.7us
    # ACT_TABLE_LOAD while DMAs are still streaming.
    scr = sb.tile([P, 4], BF16, tag="scr", bufs=1, name="scr")
    nc.scalar.activation(scr[:], ones4[:], AF.Exp, scale=SCALE)
    # Dummy matmul chatter keeps the PE activity monitor busy through the
    # DMA prologue so real matmuls start at the warm 2.4 GHz clock.
    wps = ps.tile([P, 4], F32, tag="fil", bufs=2, name="warm")
    for i in range(48):
        nc.tensor.matmul(wps[0:4, :], ones4[:], ones4[:], start=(i == 0),
                         stop=(i == 47))

    QT, KT, Vt, UN, YSB = {}, {}, {}, {}, {}

    # ---- filler units (PE work interleaved into the attention sweeps) ----
    def qk_unit(which, d, sc):
        w, xs, bias, outmap = (
            (wq, xq_t, bq, QT) if which == "q" else (wk, xk_t, bk, KT)
        )
        pg = ps.tile([P, QC], F32, tag="fil", bufs=2, name=f"pg_{which}{d}{sc}")
        for c in range(CD):
            nc.tensor.matmul(
                pg[:],
                (w[:, c, d * P : (d + 1) * P]),
                (xs[sc][:, c, :]),
                start=(c == 0),
                stop=(c == CD - 1),
            )
        t = sb.tile([P, QC], BF16, tag=f"{which}t", bufs=8, name=f"{which}T{d}{sc}")
        nc.vector.tensor_scalar_add(t[:], pg[:], bias[:, d : d + 1])
        outmap[d, sc] = t

    def v_unit(kt):
        blk, off = divmod(kt, 4)
        pg = ps.tile([P, DG], F32, tag="fil", bufs=2, name=f"pg_v{kt}")
        for c in range(CD):
            nc.tensor.matmul(
                pg[:],
                (xv_t[blk][:, c, off * P : (off + 1) * P]),
                (wv[:, c, :]),
                start=(c == 0),
                stop=(c == CD - 1),
            )
        vt = sb.tile([P, 4, DH + 1], BF16, tag="v", bufs=16, name=f"V{kt}")
        nc.vector.tensor_copy(
            vt[:, :, 0:DH], pg[:].rearrange("p (g d) -> p g d", g=4)
        )
        nc.vector.tensor_copy(vt[:, :, DH : DH + 1], ones4[:, :, None])
        Vt[kt] = vt

    def outproj_unit(qt, ec):
        qcp, qi = divmod(qt, 4)
        if ec == 0:
            YSB[qt] = sb.tile([P, D], BF16, tag="y", bufs=5, name=f"Y{qt}")
        ysb = YSB[qt]
        yp = ps.tile([P, QC], F32, tag="fil", bufs=2, name=f"yp{qt}_{ec}")
        for pr in range(2):
            nc.tensor.matmul(
                yp[:],
                (UN[qcp, pr][:, qi * P : (qi + 1) * P]),
                (woT[pr][:, ec * QC : (ec + 1) * QC]),
                start=(pr == 0),
                stop=(pr == 1),
            )
        nc.vector.tensor_copy(ysb[:, ec * QC : (ec + 1) * QC], yp[:])
        if ec == 1:
            nc.sync.dma_start(io["y"][qt * P : (qt + 1) * P, :], ysb[:])

    # Unit registry: creation order = PE priority hint; need() force-creates
    # a unit right before the sweep references its tiles (Python ordering),
    # emit() paces the rest into the sweeps as PE filler.
    unit_defs, order, done = {}, deque(), set()

    def add_unit(key, fn):
        unit_defs[key] = fn
        order.append(key)

    def run_unit(key):
        if key in done:
            return
        done.add(key)
        unit_defs[key]()

    def emit(n):
        c = 0
        while order and c < n:
            k = order.popleft()
            if k in done:
                continue
            run_unit(k)
            c += 1

    def need(key):
        if key in unit_defs:
            run_unit(key)

    for d in range(2):
        for sc in range(NSB):
            add_unit(("k", d, sc), lambda d=d, sc=sc: qk_unit("k", d, sc))
            add_unit(("q", d, sc), lambda d=d, sc=sc: qk_unit("q", d, sc))
    for kt in range(NKT):
        add_unit(("v", kt), lambda kt=kt: v_unit(kt))

    # prologue: just enough for sweep 0 to start
    for key in (("k", 0, 0), ("q", 0, 0), ("v", 0), ("v", 1), ("k", 1, 0)):
        run_unit(key)
    # re-prioritize the queue: K(d0) early, V in k-sweep order, K(d1)/Q later
    order = deque(
        [("k", 0, 1), ("q", 1, 0), ("k", 0, 2)]
        + [("v", kt) for kt in range(2, 8)]
        + [("k", 0, 3), ("k", 1, 1)]
        + [("v", kt) for kt in range(8, 16)]
        + [("k", 1, 2), ("k", 1, 3)]
        + [("q", d, sc) for sc in range(1, NSB) for d in range(2)]
    )

    # ---- attention: 8 k-sweeps of (q-chunk, head-pair) -------------------
    for qc in range(NQC):
        z4 = sb.tile([4, QC], F32, tag="z4", bufs=2, name=f"z4_{qc}")
        u64 = {}
        for pair in range(2):
            pr = pair
            need(("q", pr, qc))
            U = {
                i: ps.tile([P, QC], F32, tag="u", bufs=2, name=f"U{qc}_{pair}_{i}")
                for i in (0, 1)
            }
            for kg in range(NKT // 2):
                need(("k", pr, kg // 2))
                need(("v", kg * 2))
                need(("v", kg * 2 + 1))
                st = {
                    i: ps.tile(
                        [P, 2, QC], F32, tag="st", bufs=2, name=f"st{qc}{pair}{kg}{i}"
                    )
                    for i in (0, 1)
                }
                # scores^T: two heads on PE row-groups 0/64, adjacent issue
                # -> concurrent execution (64-contraction each).
                for kk in range(2):
                    kt_ = kg * 2 + kk
                    sc, off = divmod(kt_, 4)
                    for i in (0, 1):
                        lo = i * 64
                        nc.tensor.matmul(
                            st[i][:, kk, :],
                            (KT[pr, sc][lo : lo + 64, off * P : (off + 1) * P]),
                            (QT[pr, qc][lo : lo + 64, :]),
                            start=True,
                            stop=True,
                            tile_position=(lo, 0),
                        )
                pt = {
                    i: sb.tile(
                        [P, 2, QC], BF16, tag="pt", bufs=6, name=f"pt{qc}{pair}{kg}{i}"
                    )
                    for i in (0, 1)
                }
                for i in (0, 1):
                    nc.scalar.activation(pt[i][:], st[i][:], AF.Exp, scale=SCALE)
                for i in (0, 1):
                    h = 2 * pair + i
                    for kk in range(2):
                        kt_ = kg * 2 + kk
                        nc.tensor.matmul(
                            U[i][0:65, :],
                            (Vt[kt_][:, h, :]),
                            (pt[i][:, kk, :]),
                            start=(kg == 0 and kk == 0),
                            stop=(kg == NKT // 2 - 1 and kk == 1),
                        )
                emit(2 if (qc == 0 and pair == 0) else 1)

            # evacuate U fast so the 2 U banks recycle for the next pair
            for i in (0, 1):
                t = sb.tile([64, QC], BF16, tag="u64", bufs=4, name=f"u64_{qc}{pair}{i}")
                nc.vector.tensor_copy(t[:], U[i][0:64, :])
                u64[pair, i] = t
                zs = sb.tile([65, QC], F32, tag="zs", bufs=3, name=f"zs{qc}{pair}{i}")
                nc.vector.tensor_copy(zs[64:65, :], U[i][64:65, :])
                j = 2 * pair + i
                nc.sync.dma_start(z4[j : j + 1, :], zs[64:65, :])

        # normalization for this q-chunk: one batched reciprocal, then
        # per-head broadcast-multiply into UN [d, q] bf16.
        rz4 = sb.tile([4, QC], F32, tag="rz4", bufs=2, name=f"rz4_{qc}")
        nc.vector.reciprocal(rz4[:], z4[:])
        for pair in range(2):
            UN[qc, pair] = sb.tile([P, QC], BF16, tag="un", bufs=8, name=f"UN{qc}{pair}")
            for i in (0, 1):
                j = 2 * pair + i
                if j == 0:
                    r0 = rz4[0:1, :]
                else:
                    r0t = sb.tile([1, QC], F32, tag="r0", bufs=3, name=f"r0_{qc}_{j}")
                    nc.sync.dma_start(r0t[:], rz4[j : j + 1, :])
                    r0 = r0t[:]
                rb = sb.tile([64, QC], F32, tag="rb", bufs=4, name=f"rb{qc}{pair}{i}")
                nc.gpsimd.partition_broadcast(rb[:], r0, channels=64)
                if i == 0:
                    nc.vector.tensor_mul(UN[qc, pair][0:64, :], u64[pair, i][:], rb[:])
                else:
                    tmp = sb.tile([64, QC], BF16, tag="untmp", bufs=3, name=f"untmp{qc}{pair}")
                    nc.vector.tensor_mul(tmp[:], u64[pair, i][:], rb[:])
                    nc.sync.dma_start(UN[qc, pair][64:128, :], tmp[:])
        for qi in range(4):
            for ec in range(2):
                add_unit(
                    ("o", qc * 4 + qi, ec),
                    lambda qt=qc * 4 + qi, ec=ec: outproj_unit(qt, ec),
                )

    while order:
        emit(1)


def build_program():
    nc = bacc.Bacc(
        "TRN2", target_bir_lowering=False, debug=False, num_devices=NCORES
    )
    io = {
        "xq": nc.dram_tensor("xq", [NSB, P, CD, QC], BF16, kind="ExternalInput").ap(),
        "xk": nc.dram_tensor("xk", [NSB, P, CD, QC], BF16, kind="ExternalInput").ap(),
        "xv": nc.dram_tensor("xv", [NSB, P, CD, QC], BF16, kind="ExternalInput").ap(),
        "wq": nc.dram_tensor("wq", [P, CD, DG], BF16, kind="ExternalInput").ap(),
        "wk": nc.dram_tensor("wk", [P, CD, DG], BF16, kind="ExternalInput").ap(),
        "wv": nc.dram_tensor("wv", [P, CD, DG], BF16, kind="ExternalInput").ap(),
        "wo": nc.dram_tensor("wo", [DG, D], BF16, kind="ExternalInput").ap(),
        "bq": nc.dram_tensor("bq", [DG], F32, kind="ExternalInput").ap(),
        "bk": nc.dram_tensor("bk", [DG], F32, kind="ExternalInput").ap(),
        "ones4": nc.dram_tensor("ones4", [P, 4], BF16, kind="ExternalInput").ap(),
        "y": nc.dram_tensor("y", [S, D], BF16, kind="ExternalOutput").ap(),
    }
    with tile.TileContext(nc) as tc:
        with ExitStack() as ctx:
            _body(ctx, tc, io)
    nc.compile()
    return nc


_CACHE = {}


def _get_program():
    if "nc" not in _CACHE:
        _CACHE["nc"] = build_program()
    return _CACHE["nc"]


def make_in_maps(inputs):
    q = np.asarray(inputs["query"], np.float32)
    k = np.asarray(inputs["key"], np.float32)
    v = np.asarray(inputs["value"], np.float32)
    W_q = np.asarray(inputs["W_q"], np.float32)
    W_k = np.asarray(inputs["W_k"], np.float32)
    W_v = np.asarray(inputs["W_v"], np.float32)
    W_o = np.asarray(inputs["W_o"], np.float32)
    b_q = np.asarray(inputs["b_q"], np.float32)
    b_k = np.asarray(inputs["b_k"], np.float32)

    bf = ml_dtypes.bfloat16

    def xblocks(x, b):
        # [S, D] -> [sb, p, c, s] s-major 1MB blocks
        return np.ascontiguousarray(
            x[b].T.reshape(CD, P, NSB, QC).transpose(2, 1, 0, 3)
        ).astype(bf)

    def wblocks(W, sl):
        # W[sl, :].T -> [p, c, d]
        return np.ascontiguousarray(
            W[sl, :].T.reshape(CD, P, DG).transpose(1, 0, 2)
        ).astype(bf)

    xb = [[xblocks(x, b) for b in range(B)] for x in (q, k, v)]
    in_maps = []
    for core in range(NCORES):
        b, g = divmod(core, NG)
        sl = slice(g * DG, (g + 1) * DG)
        in_maps.append(
            {
                "xq": xb[0][b],
                "xk": xb[1][b],
                "xv": xb[2][b],
                "wq": wblocks(W_q, sl),
                "wk": wblocks(W_k, sl),
                "wv": wblocks(W_v, sl),
                "wo": np.ascontiguousarray(W_o[:, sl].T).astype(bf),
                "bq": np.ascontiguousarray(b_q[sl]),
                "bk": np.ascontiguousarray(b_k[sl]),
                "ones4": np.ones((P, 4), bf),
            }
        )
    return in_maps


def kernel(**inputs):
    from concourse.bass_utils import run_bass_kernel_spmd

    nc = _get_program()
    in_maps = make_in_maps(inputs)
    trace = bool(int(os.environ.get("MHA_TRACE", "0")))
    res = run_bass_kernel_spmd(nc, in_maps, list(range(NCORES)), trace=trace)
    _CACHE["last_results"] = res

    W_o = np.asarray(inputs["W_o"], np.float32)
    b_v = np.asarray(inputs["b_v"], np.float32)
    b_o = np.asarray(inputs["b_o"], np.float32)
    out = np.zeros((B, S, D), np.float32)
    for core in range(NCORES):
        b = core // NG
        out[b] += res.results[core]["y"].astype(np.float32)
    out += (W_o @ b_v + b_o)[None, None, :]
    return out


# revision 8
# speedup vs baseline: 1.2490x; 1.0636x over previous
"""Multi-head attention (B=2, S=2048, D=1024, H=16) on 8 Trainium2 cores.

Sharding: core = 4*b + g  (b = batch 0..1, g = head-group 0..3, 4 heads each).

Single fused pipeline per core:
  - inputs host-relaid to s-major 1MB blocks [sb, p, c, s]; all input DMA
    drains through one HWDGE FIFO at ~400GB/s with ~7us startup, so every
    DMA-gated compute unit is stamped with tile_wait_until at its realistic
    arrival time — this keeps the Tile scheduler's static per-engine order
    feasible at runtime (its own DMA model has no shared-bandwidth cap);
  - attention runs per (q-chunk, head-pair) k-sweeps, software-pipelined at
    creation: tick t emits ST+exp(t) then PV(t-1), so the next sweep's first
    scores overlap the previous sweep's last PV drain (no ACT bubble);
  - ST = scores^T via 64-contraction matmuls, the two heads of a pair issued
    adjacently on PE row-groups (0,0)/(64,0) -> concurrent execution;
  - exp on ACT is the steady-state bottleneck (128 instrs x ~1.11us);
  - PV accumulates U_h [65, q] in PSUM (row 64 = softmax denominator via a
    ones column in V_aug); U evacuated to SBUF right after each sweep so the
    2 U banks recycle; reciprocals batched per q-chunk;
  - projection / out-projection units interleave as PE filler (HAM stays
    warm); V bias folded out algebraically (host adds W_o@b_v + b_o).

All matmuls in bfloat16 (1 cycle/row, FWL weight loads); fp32 PSUM accum.
"""

import os
from contextlib import ExitStack

import ml_dtypes
import numpy as np

import concourse.bass as bass
import concourse.tile as tile
from concourse import bacc, mybir

B, S, D = 2, 2048, 1024
H, DH = 16, 64
NCORES = 8
NG = 4                  # head-group shards
DG = D // NG            # 256 dims per head-group (4 heads)
P = 128
QC = 512                # q-chunk width
NQC = S // QC           # 4
NKT = S // P            # 16 k-tiles of 128
NSB = S // QC           # 4 s-blocks per input
CD = D // P             # 8 contraction chunks
F32 = mybir.dt.float32
BF16 = mybir.dt.bfloat16
AF = mybir.ActivationFunctionType
SCALE = 1.0 / float(np.sqrt(D))

# DMA arrival estimates (us): single FIFO queue, ~0.4 MB/us, ~7us startup.
_T0, _RATE = 7.0, 0.4


def _body(ctx: ExitStack, tc: "tile.TileContext", io: dict):
    nc = tc.nc
    ctx.enter_context(nc.allow_low_precision(reason="bf16 matmul pipeline"))
    sb = ctx.enter_context(tc.tile_pool(name="sb", bufs=1))
    ps = ctx.enter_context(tc.tile_pool(name="ps", bufs=1, space="PSUM"))

    # ---- input DMAs in FIFO priority order; track cumulative-MB ETAs ------
    eta = {}
    cum = [0.0]

    def ldma(nm, shape, dt, src, mb, nsplit=1):
        t = sb.tile(shape, dt, tag=nm, bufs=1, name=nm)
        if nsplit == 1:
            nc.sync.dma_start(t[:], src)
        else:
            step = shape[1] // nsplit
            for i in range(nsplit):
                sl = slice(i * step, (i + 1) * step)
                nc.sync.dma_start(t[:, sl], src[:, sl])
        cum[0] += mb
        eta[nm] = _T0 + cum[0] / _RATE
        return t

    ones4 = ldma("ones4", [P, 4], BF16, io["ones4"], 0.01)
    wk = ldma("wk", [P, CD, DG], BF16, io["wk"], 0.5)
    xk_t = [ldma("xk0", [P, CD, QC], BF16, io["xk"][0], 1.0, nsplit=2)]
    wq = ldma("wq", [P, CD, DG], BF16, io["wq"], 0.5)
    xq_t = [ldma("xq0", [P, CD, QC], BF16, io["xq"][0], 1.0, nsplit=2)]
    bk = ldma("bk", [P, 2], F32, io["bk"], 0.01)
    bq = ldma("bq", [P, 2], F32, io["bq"], 0.01)
    for b_ in range(1, NSB):
        xk_t.append(ldma(f"xk{b_}", [P, CD, QC], BF16, io["xk"][b_], 1.0))
    wv = ldma("wv", [P, CD, DG], BF16, io["wv"], 0.5)
    xv_t = [ldma(f"xv{b_}", [P, CD, QC], BF16, io["xv"][b_], 1.0) for b_ in range(NSB)]
    for b_ in range(1, NSB):
        xq_t.append(ldma(f"xq{b_}", [P, CD, QC], BF16, io["xq"][b_], 1.0))
    woT = [
        ldma(f"wo{pr}", [P, D], BF16, io["wo"][pr * P : (pr + 1) * P, :], 0.25)
        for pr in range(2)
    ]

    # ACT table preload: tiny exp on the first-arriving tile triggers the
    # one-time ~2.7us ACT_TABLE_LOAD while DMAs are still streaming.
    scr = sb.tile([P, 4], BF16, tag="scr", bufs=1, name="scr")
    nc.scalar.activation(scr[:], ones4[:], AF.Exp, scale=SCALE)

    QT, KT, Vt, UN, YSB = {}, {}, {}, {}, {}

    # ---- PE filler units, stamped with realistic DMA-arrival times --------
    def qk_unit(which, d, sc):
        w, xs, bias, outmap = (
            (wq, xq_t, bq, QT) if which == "q" else (wk, xk_t, bk, KT)
        )
        pg = ps.tile([P, QC], F32, tag="fil", bufs=2, name=f"pg_{which}{d}{sc}")
        for c in range(CD):
            nc.tensor.matmul(
                pg[:],
                (w[:, c, d * P : (d + 1) * P]),
                (xs[sc][:, c, :]),
                start=(c == 0),
                stop=(c == CD - 1),
            )
        t = sb.tile([P, QC], BF16, tag=f"{which}t", bufs=8, name=f"{which}T{d}{sc}")
        nc.vector.tensor_scalar_add(t[:], pg[:], bias[:, d : d + 1])
        outmap[d, sc] = t

    def v_unit(kt):
        blk, off = divmod(kt, 4)
        pg = ps.tile([P, DG], F32, tag="fil", bufs=2, name=f"pg_v{kt}")
        for c in range(CD):
            nc.tensor.matmul(
                pg[:],
                (xv_t[blk][:, c, off * P : (off + 1) * P]),
                (wv[:, c, :]),
                start=(c == 0),
                stop=(c == CD - 1),
            )
        vt = sb.tile([P, 4, DH + 1], BF16, tag="v", bufs=16, name=f"V{kt}")
        nc.vector.tensor_copy(
            vt[:, :, 0:DH], pg[:].rearrange("p (g d) -> p g d", g=4)
        )
        nc.vector.tensor_copy(vt[:, :, DH : DH + 1], ones4[:, :, None])
        Vt[kt] = vt

    def outproj_unit(qt, ec):
        qcp, qi = divmod(qt, 4)
        if ec == 0:
            YSB[qt] = sb.tile([P, D], BF16, tag="y", bufs=4, name=f"Y{qt}")
        ysb = YSB[qt]
        yp = ps.tile([P, QC], F32, tag="fil", bufs=2, name=f"yp{qt}_{ec}")
        for pr in range(2):
            nc.tensor.matmul(
                yp[:],
                (UN[qcp, pr][:, qi * P : (qi + 1) * P]),
                (woT[pr][:, ec * QC : (ec + 1) * QC]),
                start=(pr == 0),
                stop=(pr == 1),
            )
        nc.vector.tensor_copy(ysb[:, ec * QC : (ec + 1) * QC], yp[:])
        if ec == 1:
            nc.sync.dma_start(io["y"][qt * P : (qt + 1) * P, :], ysb[:])

    def stamped(us, fn):
        with tc.tile_wait_until(us / 1000.0):
            fn()

    # create all projection units upfront; wait-stamps place them in the
    # scheduler's timeline at their realistic earliest-start times.
    stamped(eta["xk0"], lambda: qk_unit("k", 0, 0))
    stamped(eta["xq0"], lambda: qk_unit("q", 0, 0))
    stamped(eta["xk0"] + 4.0, lambda: qk_unit("k", 1, 0))
    stamped(eta["xq0"] + 4.0, lambda: qk_unit("q", 1, 0))
    for sc in range(1, NSB):
        stamped(eta[f"xk{sc}"], lambda sc=sc: qk_unit("k", 0, sc))
        stamped(eta[f"xk{sc}"] + 4.0, lambda sc=sc: qk_unit("k", 1, sc))
    for kt in range(NKT):
        stamped(eta[f"xv{kt // 4}"] + 0.2, lambda kt=kt: v_unit(kt))
    for qc in range(1, NQC):
        for d in range(2):
            stamped(eta[f"xq{qc}"] + 0.2, lambda d=d, qc=qc: qk_unit("q", d, qc))

    # ---- attention: 8 software-pipelined k-sweeps -------------------------
    sweeps = [(qc, pair) for qc in range(NQC) for pair in range(2)]
    z4 = {}
    u64 = {}
    prev = None  # (qc, pair, U, pt, kg) awaiting its PV + (for kg=7) evac

    def do_pv(qc, pair, U, pt, kg):
        for i in (0, 1):
            h = 2 * pair + i
            for kk in range(2):
                kt_ = kg * 2 + kk
                nc.tensor.matmul(
                    U[i][0:65, :],
                    (Vt[kt_][:, h, :]),
                    (pt[i][:, kk, :]),
                    start=(kg == 0 and kk == 0),
                    stop=(kg == NKT // 2 - 1 and kk == 1),
                )

    def do_evac(qc, pair, U):
        for i in (0, 1):
            zs = sb.tile([65, QC], F32, tag="zs", bufs=2, name=f"zs{qc}{pair}{i}")
            nc.vector.tensor_copy(zs[64:65, :], U[i][64:65, :])
            j = 2 * pair + i
            nc.sync.dma_start(z4[qc][j : j + 1, :], zs[64:65, :])
        for i in (0, 1):
            t = sb.tile([64, QC], BF16, tag="u64", bufs=4, name=f"u64_{qc}{pair}{i}")
            nc.vector.tensor_copy(t[:], U[i][0:64, :])
            u64[pair, i] = t

    def do_norm(qc):
        rz4 = sb.tile([4, QC], BF16, tag="rz4", bufs=2, name=f"rz4_{qc}")
        nc.vector.reciprocal(rz4[:], z4[qc][:])
        for pair in range(2):
            UN[qc, pair] = sb.tile(
                [P, QC], BF16, tag="un", bufs=8, name=f"UN{qc}{pair}"
            )
            for i in (0, 1):
                j = 2 * pair + i
                if j == 0:
                    r0 = rz4[0:1, :]
                else:
                    r0t = sb.tile([1, QC], BF16, tag="r0", bufs=3, name=f"r0_{qc}_{j}")
                    nc.sync.dma_start(r0t[:], rz4[j : j + 1, :])
                    r0 = r0t[:]
                rb = sb.tile([64, QC], BF16, tag="rb", bufs=4, name=f"rb{qc}{pair}{i}")
                nc.gpsimd.partition_broadcast(rb[:], r0, channels=64)
                if i == 0:
                    nc.vector.tensor_mul(UN[qc, pair][0:64, :], u64[pair, i][:], rb[:])
                else:
                    tmp = sb.tile(
                        [64, QC], BF16, tag="untmp", bufs=2, name=f"untmp{qc}{pair}"
                    )
                    nc.vector.tensor_mul(tmp[:], u64[pair, i][:], rb[:])
                    nc.sync.dma_start(UN[qc, pair][64:128, :], tmp[:])
        for qi in range(4):
            for ec in range(2):
                outproj_unit(qc * 4 + qi, ec)

    for qc, pair in sweeps:
        pr = pair
        if pair == 0:
            z4[qc] = sb.tile([4, QC], F32, tag="z4", bufs=2, name=f"z4_{qc}")
        U = {
            i: ps.tile([P, QC], F32, tag="u", bufs=2, name=f"U{qc}_{pair}_{i}")
            for i in (0, 1)
        }
        for kg in range(NKT // 2):
            st = {
                i: ps.tile(
                    [P, 2, QC], F32, tag="st", bufs=2, name=f"st{qc}{pair}{kg}{i}"
                )
                for i in (0, 1)
            }
            # scores^T: two heads on PE row-groups 0/64, adjacent issue ->
            # concurrent execution (64-contraction each).
            for kk in range(2):
                kt_ = kg * 2 + kk
                sc, off = divmod(kt_, 4)
                for i in (0, 1):
                    lo = i * 64
                    nc.tensor.matmul(
                        st[i][:, kk, :],
                        (KT[pr, sc][lo : lo + 64, off * P : (off + 1) * P]),
                        (QT[pr, qc][lo : lo + 64, :]),
                        start=True,
                        stop=True,
                        tile_position=(lo, 0),
                    )
            pt = {
                i: sb.tile(
                    [P, 2, QC], BF16, tag="pt", bufs=9, name=f"pt{qc}{pair}{kg}{i}"
                )
                for i in (0, 1)
            }
            for i in (0, 1):
                nc.scalar.activation(pt[i][:], st[i][:], AF.Exp, scale=SCALE)
            # previous tick's PV (pipelined one tick behind ST/exp)
            if prev is not None:
                pqc, ppair, pU, ppt, pkg = prev
                do_pv(pqc, ppair, pU, ppt, pkg)
                if pkg == NKT // 2 - 1:
                    do_evac(pqc, ppair, pU)
                    if ppair == 1:
                        do_norm(pqc)
            prev = (qc, pair, U, pt, kg)

    # drain: final PV, evac, norm, out-projection of the last q-chunk
    pqc, ppair, pU, ppt, pkg = prev
    do_pv(pqc, ppair, pU, ppt, pkg)
    do_evac(pqc, ppair, pU)
    do_norm(pqc)


def build_program():
    nc = bacc.Bacc(
        "TRN2", target_bir_lowering=False, debug=False, num_devices=NCORES
    )
    io = {
        "xq": nc.dram_tensor("xq", [NSB, P, CD, QC], BF16, kind="ExternalInput").ap(),
        "xk": nc.dram_tensor("xk", [NSB, P, CD, QC], BF16, kind="ExternalInput").ap(),
        "xv": nc.dram_tensor("xv", [NSB, P, CD, QC], BF16, kind="ExternalInput").ap(),
        "wq": nc.dram_tensor("wq", [P, CD, DG], BF16, kind="ExternalInput").ap(),
        "wk": nc.dram_tensor("wk", [P, CD, DG], BF16, kind="ExternalInput").ap(),
        "wv": nc.dram_tensor("wv", [P, CD, DG], BF16, kind="ExternalInput").ap(),
        "wo": nc.dram_tensor("wo", [DG, D], BF16, kind="ExternalInput").ap(),
        "bq": nc.dram_tensor("bq", [P, 2], F32, kind="ExternalInput").ap(),
        "bk": nc.dram_tensor("bk", [P, 2], F32, kind="ExternalInput").ap(),
        "ones4": nc.dram_tensor("ones4", [P, 4], BF16, kind="ExternalInput").ap(),
        "y": nc.dram_tensor("y", [S, D], BF16, kind="ExternalOutput").ap(),
    }
    with tile.TileContext(nc) as tc:
        with ExitStack() as ctx:
            _body(ctx, tc, io)
    nc.compile()
    return nc


_CACHE = {}


def _get_program():
    if "nc" not in _CACHE:
        _CACHE["nc"] = build_program()
    return _CACHE["nc"]


def make_in_maps(inputs):
    q = np.asarray(inputs["query"], np.float32)
    k = np.asarray(inputs["key"], np.float32)
    v = np.asarray(inputs["value"], np.float32)
    W_q = np.asarray(inputs["W_q"], np.float32)
    W_k = np.asarray(inputs["W_k"], np.float32)
    W_v = np.asarray(inputs["W_v"], np.float32)
    W_o = np.asarray(inputs["W_o"], np.float32)
    b_q = np.asarray(inputs["b_q"], np.float32)
    b_k = np.asarray(inputs["b_k"], np.float32)

    bf = ml_dtypes.bfloat16

    def xblocks(x, b):
        # [S, D] -> [sb, p, c, s] s-major 1MB blocks
        return np.ascontiguousarray(
            x[b].T.reshape(CD, P, NSB, QC).transpose(2, 1, 0, 3)
        ).astype(bf)

    def wblocks(W, sl):
        # W[sl, :].T -> [p, c, d]
        return np.ascontiguousarray(
            W[sl, :].T.reshape(CD, P, DG).transpose(1, 0, 2)
        ).astype(bf)

    def bblocks(bvec, sl):
        # [DG] -> [p, t]: bias for dim t*128+p
        return np.ascontiguousarray(bvec[sl].reshape(2, P).T)

    xb = [[xblocks(x, b) for b in range(B)] for x in (q, k, v)]
    in_maps = []
    for core in range(NCORES):
        b, g = divmod(core, NG)
        sl = slice(g * DG, (g + 1) * DG)
        in_maps.append(
            {
                "xq": xb[0][b],
                "xk": xb[1][b],
                "xv": xb[2][b],
                "wq": wblocks(W_q, sl),
                "wk": wblocks(W_k, sl),
                "wv": wblocks(W_v, sl),
                "wo": np.ascontiguousarray(W_o[:, sl].T).astype(bf),
                "bq": bblocks(b_q, sl),
                "bk": bblocks(b_k, sl),
                "ones4": np.ones((P, 4), bf),
            }
        )
    return in_maps


def kernel(**inputs):
    from concourse.bass_utils import run_bass_kernel_spmd

    nc = _get_program()
    in_maps = make_in_maps(inputs)
    trace = bool(int(os.environ.get("MHA_TRACE", "0")))
    res = run_bass_kernel_spmd(nc, in_maps, list(range(NCORES)), trace=trace)
    _CACHE["last_results"] = res

    W_o = np.asarray(inputs["W_o"], np.float32)
    b_v = np.asarray(inputs["b_v"], np.float32)
    b_o = np.asarray(inputs["b_o"], np.float32)
    out = np.zeros((B, S, D), np.float32)
    for core in range(NCORES):
        b = core // NG
        out[b] += res.results[core]["y"].astype(np.float32)
    out += (W_o @ b_v + b_o)[None, None, :]
    return out


# revision 15
# speedup vs baseline: 1.2986x; 1.0398x over previous
"""Multi-head attention (B=2, S=2048, D=1024, H=16) on 8 Trainium2 cores.

Sharding: core = 4*b + g  (b = batch 0..1, g = head-group 0..3, 4 heads each).

Single fused pipeline per core:
  - inputs host-relaid to s-major 1MB blocks [sb, p, c, s]; all input DMA
    drains through one HWDGE FIFO at ~400GB/s with ~7us startup, so every
    DMA-gated compute unit is stamped with tile_wait_until at its realistic
    arrival time — this keeps the Tile scheduler's static per-engine order
    feasible at runtime (its own DMA model has no shared-bandwidth cap);
  - attention runs per (q-chunk, head-pair) k-sweeps, software-pipelined at
    creation: tick t emits ST+exp(t) then PV(t-1), so the next sweep's first
    scores overlap the previous sweep's last PV drain (no ACT bubble);
  - ST = scores^T via 64-contraction matmuls, the two heads of a pair issued
    adjacently on PE row-groups (0,0)/(64,0) -> concurrent execution;
  - exp on ACT is the steady-state bottleneck (128 instrs x ~1.11us);
  - PV accumulates U_h [65, q] in PSUM (row 64 = softmax denominator via a
    ones column in V_aug); U evacuated to SBUF right after each sweep so the
    2 U banks recycle; reciprocals batched per q-chunk;
  - projection / out-projection units interleave as PE filler (HAM stays
    warm); V bias folded out algebraically (host adds W_o@b_v + b_o).

All matmuls in bfloat16 (1 cycle/row, FWL weight loads); fp32 PSUM accum.
"""

import os
from contextlib import ExitStack

import ml_dtypes
import numpy as np

import concourse.bass as bass
import concourse.tile as tile
from concourse import bacc, mybir

B, S, D = 2, 2048, 1024
H, DH = 16, 64
NCORES = 8
NG = 4                  # head-group shards
DG = D // NG            # 256 dims per head-group (4 heads)
P = 128
QC = 512                # q-chunk width
NQC = S // QC           # 4
NKT = S // P            # 16 k-tiles of 128
NSB = S // QC           # 4 s-blocks per input
CD = D // P             # 8 contraction chunks
F32 = mybir.dt.float32
BF16 = mybir.dt.bfloat16
AF = mybir.ActivationFunctionType
SCALE = 1.0 / float(np.sqrt(D))

# DMA arrival estimates (us): single FIFO queue, ~0.4 MB/us, ~7us startup.
_T0, _RATE = 7.0, 0.4


def _body(ctx: ExitStack, tc: "tile.TileContext", io: dict):
    nc = tc.nc
    ctx.enter_context(nc.allow_low_precision(reason="bf16 matmul pipeline"))
    sb = ctx.enter_context(tc.tile_pool(name="sb", bufs=1))
    ps = ctx.enter_context(tc.tile_pool(name="ps", bufs=1, space="PSUM"))

    # ---- input DMAs in FIFO priority order; track cumulative-MB ETAs ------
    eta = {}
    cum = [0.0]

    def ldma(nm, shape, dt, src, mb, nsplit=1):
        t = sb.tile(shape, dt, tag=nm, bufs=1, name=nm)
        if nsplit == 1:
            nc.sync.dma_start(t[:], src)
        else:
            step = shape[1] // nsplit
            for i in range(nsplit):
                sl = slice(i * step, (i + 1) * step)
                nc.sync.dma_start(t[:, sl], src[:, sl])
        cum[0] += mb
        eta[nm] = _T0 + cum[0] / _RATE
        return t

    ones4 = ldma("ones4", [P, 4], BF16, io["ones4"], 0.01)
    wk = ldma("wk", [P, CD, DG], BF16, io["wk"], 0.5)
    xk_t = [ldma("xk0", [P, CD, QC], BF16, io["xk"][0], 1.0, nsplit=2)]
    wq = ldma("wq", [P, CD, DG], BF16, io["wq"], 0.5)
    xq_t = [ldma("xq0", [P, CD, QC], BF16, io["xq"][0], 1.0, nsplit=2)]
    bk = ldma("bk", [P, 2], F32, io["bk"], 0.01)
    bq = ldma("bq", [P, 2], F32, io["bq"], 0.01)
    for b_ in range(1, NSB):
        xk_t.append(ldma(f"xk{b_}", [P, CD, QC], BF16, io["xk"][b_], 1.0))
    wv = ldma("wv", [P, CD, DG], BF16, io["wv"], 0.5)
    xv_t = [ldma(f"xv{b_}", [P, CD, QC], BF16, io["xv"][b_], 1.0) for b_ in range(NSB)]
    for b_ in range(1, NSB):
        xq_t.append(ldma(f"xq{b_}", [P, CD, QC], BF16, io["xq"][b_], 1.0))
    woT = [
        ldma(f"wo{pr}", [P, D], BF16, io["wo"][pr * P : (pr + 1) * P, :], 0.25)
        for pr in range(2)
    ]

    # ACT table preload: tiny exp on the first-arriving tile triggers the
    # one-time ~2.7us ACT_TABLE_LOAD while DMAs are still streaming.
    scr = sb.tile([P, 4], BF16, tag="scr", bufs=1, name="scr")
    nc.scalar.activation(scr[:], ones4[:], AF.Exp, scale=SCALE)

    QT, KT, Vt, UN, YSB = {}, {}, {}, {}, {}

    # ---- PE filler units, stamped with realistic DMA-arrival times --------
    def qk_unit(which, d, sc):
        w, xs, bias, outmap = (
            (wq, xq_t, bq, QT) if which == "q" else (wk, xk_t, bk, KT)
        )
        pg = ps.tile([P, QC], F32, tag="fil", bufs=2, name=f"pg_{which}{d}{sc}")
        for c in range(CD):
            nc.tensor.matmul(
                pg[:],
                (w[:, c, d * P : (d + 1) * P]),
                (xs[sc][:, c, :]),
                start=(c == 0),
                stop=(c == CD - 1),
            )
        t = sb.tile([P, QC], BF16, tag=f"{which}t", bufs=8, name=f"{which}T{d}{sc}")
        nc.vector.tensor_scalar_add(t[:], pg[:], bias[:, d : d + 1])
        outmap[d, sc] = t

    def v_unit(kt):
        blk, off = divmod(kt, 4)
        pg = ps.tile([P, DG], F32, tag="fil", bufs=2, name=f"pg_v{kt}")
        for c in range(CD):
            nc.tensor.matmul(
                pg[:],
                (xv_t[blk][:, c, off * P : (off + 1) * P]),
                (wv[:, c, :]),
                start=(c == 0),
                stop=(c == CD - 1),
            )
        vt = sb.tile([P, 4, DH + 1], BF16, tag="v", bufs=16, name=f"V{kt}")
        nc.vector.tensor_copy(
            vt[:, :, 0:DH], pg[:].rearrange("p (g d) -> p g d", g=4)
        )
        nc.vector.tensor_copy(vt[:, :, DH : DH + 1], ones4[:, :, None])
        Vt[kt] = vt

    def outproj_unit(qt, ec):
        qcp, qi = divmod(qt, 4)
        if ec == 0:
            YSB[qt] = sb.tile([P, D], BF16, tag="y", bufs=4, name=f"Y{qt}")
        ysb = YSB[qt]
        yp = ps.tile([P, QC], F32, tag="fil", bufs=2, name=f"yp{qt}_{ec}")
        for pr in range(2):
            nc.tensor.matmul(
                yp[:],
                (UN[qcp, pr][:, qi * P : (qi + 1) * P]),
                (woT[pr][:, ec * QC : (ec + 1) * QC]),
                start=(pr == 0),
                stop=(pr == 1),
            )
        nc.vector.tensor_copy(ysb[:, ec * QC : (ec + 1) * QC], yp[:])
        if ec == 1:
            nc.sync.dma_start(io["y"][qt * P : (qt + 1) * P, :], ysb[:])

    def stamped(us, fn):
        with tc.tile_wait_until(us / 1000.0):
            fn()

    # create all projection units upfront; wait-stamps place them in the
    # scheduler's timeline at their realistic earliest-start times.
    stamped(eta["xk0"], lambda: qk_unit("k", 0, 0))
    stamped(eta["xq0"], lambda: qk_unit("q", 0, 0))
    stamped(eta["xk0"] + 0.5, lambda: qk_unit("k", 1, 0))
    stamped(eta["xq0"] + 0.5, lambda: qk_unit("q", 1, 0))
    for sc in range(1, NSB):
        stamped(eta[f"xk{sc}"], lambda sc=sc: qk_unit("k", 0, sc))
        stamped(eta[f"xk{sc}"] + 0.5, lambda sc=sc: qk_unit("k", 1, sc))
    for kt in range(NKT):
        stamped(eta[f"xv{kt // 4}"] + 0.2, lambda kt=kt: v_unit(kt))
    for qc in range(1, NQC):
        for d in range(2):
            stamped(eta[f"xq{qc}"] + 0.2, lambda d=d, qc=qc: qk_unit("q", d, qc))

    # ---- attention: 8 software-pipelined k-sweeps -------------------------
    sweeps = [(qc, pair) for qc in range(NQC) for pair in range(2)]
    z4 = {}
    u64 = {}
    prev = None  # (qc, pair, U, pt, kg) awaiting its PV + (for kg=7) evac

    def do_pv(qc, pair, U, pt, kg):
        for i in (0, 1):
            h = 2 * pair + i
            for kk in range(2):
                kt_ = kg * 2 + kk
                nc.tensor.matmul(
                    U[i][0:65, :],
                    (Vt[kt_][:, h, :]),
                    (pt[i][:, kk, :]),
                    start=(kg == 0 and kk == 0),
                    stop=(kg == NKT // 2 - 1 and kk == 1),
                )

    def do_evac(qc, pair, U):
        for i in (0, 1):
            zs = sb.tile([65, QC], F32, tag="zs", bufs=2, name=f"zs{qc}{pair}{i}")
            nc.vector.tensor_copy(zs[64:65, :], U[i][64:65, :])
            j = 2 * pair + i
            nc.sync.dma_start(z4[qc][j : j + 1, :], zs[64:65, :])
        for i in (0, 1):
            t = sb.tile([64, QC], BF16, tag="u64", bufs=4, name=f"u64_{qc}{pair}{i}")
            nc.vector.tensor_copy(t[:], U[i][0:64, :])
            u64[pair, i] = t

    def do_norm(qc):
        rz4 = sb.tile([4, QC], BF16, tag="rz4", bufs=2, name=f"rz4_{qc}")
        nc.vector.reciprocal(rz4[:], z4[qc][:])
        for pair in range(2):
            UN[qc, pair] = sb.tile([P, QC], BF16, tag="un", bufs=8, name=f"UN{qc}{pair}")
            for i in (0, 1):
                j = 2 * pair + i
                if j == 0:
                    r0 = rz4[0:1, :]
                else:
                    r0t = sb.tile([1, QC], BF16, tag="r0", bufs=3, name=f"r0_{qc}_{j}")
                    nc.sync.dma_start(r0t[:], rz4[j : j + 1, :])
                    r0 = r0t[:]
                rb = sb.tile([64, QC], BF16, tag="rb", bufs=4, name=f"rb{qc}{pair}{i}")
                nc.gpsimd.partition_broadcast(rb[:], r0, channels=64)
                if i == 0:
                    nc.vector.tensor_mul(UN[qc, pair][0:64, :], u64[pair, i][:], rb[:])
                else:
                    tmp = sb.tile(
                        [64, QC], BF16, tag="untmp", bufs=2, name=f"untmp{qc}{pair}"
                    )
                    nc.vector.tensor_mul(tmp[:], u64[pair, i][:], rb[:])
                    nc.sync.dma_start(UN[qc, pair][64:128, :], tmp[:])

    HIPRI = 1_000_000

    for qc, pair in sweeps:
        pr = pair
        if pair == 0:
            z4[qc] = sb.tile([4, QC], F32, tag="z4", bufs=2, name=f"z4_{qc}")
        U = {
            i: ps.tile([P, QC], F32, tag="u", bufs=2, name=f"U{qc}_{pair}_{i}")
            for i in (0, 1)
        }
        for kg in range(NKT // 2):
            norm_qc = None
            with tc.high_priority(offset=HIPRI):
                st = {
                    i: ps.tile(
                        [P, 2, QC], F32, tag="st", bufs=2, name=f"st{qc}{pair}{kg}{i}"
                    )
                    for i in (0, 1)
                }
                # scores^T: two heads on PE row-groups 0/64, adjacent issue ->
                # concurrent execution (64-contraction each).
                for kk in range(2):
                    kt_ = kg * 2 + kk
                    sc, off = divmod(kt_, 4)
                    for i in (0, 1):
                        lo = i * 64
                        nc.tensor.matmul(
                            st[i][:, kk, :],
                            (KT[pr, sc][lo : lo + 64, off * P : (off + 1) * P]),
                            (QT[pr, qc][lo : lo + 64, :]),
                            start=True,
                            stop=True,
                            tile_position=(lo, 0),
                        )
                pt = {
                    i: sb.tile(
                        [P, 2, QC], BF16, tag="pt", bufs=9, name=f"pt{qc}{pair}{kg}{i}"
                    )
                    for i in (0, 1)
                }
                for i in (0, 1):
                    nc.scalar.activation(pt[i][:], st[i][:], AF.Exp, scale=SCALE)
                # previous tick's PV (pipelined one tick behind ST/exp)
                if prev is not None:
                    pqc, ppair, pU, ppt, pkg = prev
                    do_pv(pqc, ppair, pU, ppt, pkg)
                    if pkg == NKT // 2 - 1:
                        do_evac(pqc, ppair, pU)
                        if ppair == 1:
                            do_norm(pqc)
                            norm_qc = pqc
                prev = (qc, pair, U, pt, kg)
            if norm_qc is not None:
                for qi in range(4):
                    for ec in range(2):
                        outproj_unit(norm_qc * 4 + qi, ec)

    # drain: final PV, evac, norm, out-projection of the last q-chunk
    pqc, ppair, pU, ppt, pkg = prev
    with tc.high_priority(offset=HIPRI):
        do_pv(pqc, ppair, pU, ppt, pkg)
        do_evac(pqc, ppair, pU)
        do_norm(pqc)
    for qi in range(4):
        for ec in range(2):
            outproj_unit(pqc * 4 + qi, ec)


def build_program():
    nc = bacc.Bacc(
        "TRN2", target_bir_lowering=False, debug=False, num_devices=NCORES
    )
    io = {
        "xq": nc.dram_tensor("xq", [NSB, P, CD, QC], BF16, kind="ExternalInput").ap(),
        "xk": nc.dram_tensor("xk", [NSB, P, CD, QC], BF16, kind="ExternalInput").ap(),
        "xv": nc.dram_tensor("xv", [NSB, P, CD, QC], BF16, kind="ExternalInput").ap(),
        "wq": nc.dram_tensor("wq", [P, CD, DG], BF16, kind="ExternalInput").ap(),
        "wk": nc.dram_tensor("wk", [P, CD, DG], BF16, kind="ExternalInput").ap(),
        "wv": nc.dram_tensor("wv", [P, CD, DG], BF16, kind="ExternalInput").ap(),
        "wo": nc.dram_tensor("wo", [DG, D], BF16, kind="ExternalInput").ap(),
        "bq": nc.dram_tensor("bq", [P, 2], F32, kind="ExternalInput").ap(),
        "bk": nc.dram_tensor("bk", [P, 2], F32, kind="ExternalInput").ap(),
        "ones4": nc.dram_tensor("ones4", [P, 4], BF16, kind="ExternalInput").ap(),
        "y": nc.dram_tensor("y", [S, D], BF16, kind="ExternalOutput").ap(),
    }
    with tile.TileContext(nc) as tc:
        with ExitStack() as ctx:
            _body(ctx, tc, io)
    nc.compile()
    return nc


_CACHE = {}


def _get_program():
    if "nc" not in _CACHE:
        _CACHE["nc"] = build_program()
    return _CACHE["nc"]


def make_in_maps(inputs):
    q = np.asarray(inputs["query"], np.float32)
    k = np.asarray(inputs["key"], np.float32)
    v = np.asarray(inputs["value"], np.float32)
    W_q = np.asarray(inputs["W_q"], np.float32)
    W_k = np.asarray(inputs["W_k"], np.float32)
    W_v = np.asarray(inputs["W_v"], np.float32)
    W_o = np.asarray(inputs["W_o"], np.float32)
    b_q = np.asarray(inputs["b_q"], np.float32)
    b_k = np.asarray(inputs["b_k"], np.float32)

    bf = ml_dtypes.bfloat16

    def xblocks(x, b):
        # [S, D] -> [sb, p, c, s] s-major 1MB blocks
        return np.ascontiguousarray(
            x[b].T.reshape(CD, P, NSB, QC).transpose(2, 1, 0, 3)
        ).astype(bf)

    def wblocks(W, sl):
        # W[sl, :].T -> [p, c, d]
        return np.ascontiguousarray(
            W[sl, :].T.reshape(CD, P, DG).transpose(1, 0, 2)
        ).astype(bf)

    def bblocks(bvec, sl):
        # [DG] -> [p, t]: bias for dim t*128+p
        return np.ascontiguousarray(bvec[sl].reshape(2, P).T)

    xb = [[xblocks(x, b) for b in range(B)] for x in (q, k, v)]
    in_maps = []
    for core in range(NCORES):
        b, g = divmod(core, NG)
        sl = slice(g * DG, (g + 1) * DG)
        in_maps.append(
            {
                "xq": xb[0][b],
                "xk": xb[1][b],
                "xv": xb[2][b],
                "wq": wblocks(W_q, sl),
                "wk": wblocks(W_k, sl),
                "wv": wblocks(W_v, sl),
                "wo": np.ascontiguousarray(W_o[:, sl].T).astype(bf),
                "bq": bblocks(b_q, sl),
                "bk": bblocks(b_k, sl),
                "ones4": np.ones((P, 4), bf),
            }
        )
    return in_maps


def kernel(**inputs):
    from concourse.bass_utils import run_bass_kernel_spmd

    nc = _get_program()
    in_maps = make_in_maps(inputs)
    trace = bool(int(os.environ.get("MHA_TRACE", "0")))
    res = run_bass_kernel_spmd(nc, in_maps, list(range(NCORES)), trace=trace)
    _CACHE["last_results"] = res

    W_o = np.asarray(inputs["W_o"], np.float32)
    b_v = np.asarray(inputs["b_v"], np.float32)
    b_o = np.asarray(inputs["b_o"], np.float32)
    out = np.zeros((B, S, D), np.float32)
    for core in range(NCORES):
        b = core // NG
        out[b] += res.results[core]["y"].astype(np.float32)
    out += (W_o @ b_v + b_o)[None, None, :]
    return out
